# revision 16
# baseline (speedup 1.0000x reference)
"""Grouped 2-layer MLP (ConvNN) Trainium2 kernel, v2.

Math (per group g of SIZE=2048):
    h[b,g,:]   = LeakyReLU_0.2(W0[g] @ x[b] + b0[g])     (64 -> 64)
    out[b,g,:] = W1[g] @ h[b,g,:] + b1[g]                (64 -> 64)

Strategy (v2 — pipelined, fp16 output):
  - Shard the group axis over 8 cores (256 groups/core = 128 pairs),
    fully independent.
  - Groups processed in pairs stacked on the 128 partitions.  Layer 0
    contracts over x (shared by all groups): stationary [K=64, M=128]
    compact weights, no zeros.  Layer 1 contracts per group: stationary
    is a host-built 128x128 block-diagonal tile (zeros cost nothing,
    matmul time is N-driven).
  - All weights land in SBUF via chunked DMAs straight into their final
    layout (no on-chip block building, no ACT/DVE copy work).
  - Per pair: h and out live in [128, 1024] fp32 PSUM tiles (2 banks
    each; 2 bufs per tag = all 8 banks).  One ACT Prelu (+b0, fp16 out)
    evacuates h; one DVE tensor_scalar (+b1, fp16 out) evacuates out.
    Large 1024-elem instructions amortize the fixed ACT/DVE overheads.
  - Explicit 2-stage software pipeline: iteration t issues layer-0 of
    pair t and layer-1 of pair t-2, so the PE never waits on the ACT
    evacuation of the same pair (the baseline's 1.3us/pair stall) and
    stays busy enough to hold the HAM clock gate at 2.4 GHz.
  - Output is written fp16 as [pair, 128, B] (256KB contiguous DMA per
    pair) and un-transposed/cast on the host.  Input DMAs ride SWDGE
    (gpsimd) so the HWDGE sync ring is dedicated to output stores.
"""

from contextlib import ExitStack

import ml_dtypes
import numpy as np

import concourse.bass as bass
import concourse.mybir as mybir
import concourse.tile as tile
from concourse.bass_utils import run_bass_kernel_spmd

B = 1024
IN_DIM = 64
SIZE = 2048
D1 = 64
D2 = 64
NEG_SLOPE = 0.2
N_CORES = 8
GPC = SIZE // N_CORES  # 256 groups per core
NPAIR = GPC // 2  # 128 group-pairs per core
WCH = 4  # weight DMA chunks per layer
LAG = 3  # software-pipeline distance between layer 0 and layer 1
ACT_EVAC_MOD = 10**9  # disabled: PE-bound, ACT/DVE have slack
BF16 = ml_dtypes.bfloat16

_NC_CACHE = None


def _build():
    global _NC_CACHE
    if _NC_CACHE is not None:
        return _NC_CACHE

    f32 = mybir.dt.float32
    f16 = mybir.dt.float16
    bf = mybir.dt.bfloat16

    nc = bass.Bass()
    xt = nc.declare_dram_parameter("xt", [IN_DIM, B], f16, isOutput=False)
    w0c = nc.declare_dram_parameter("w0c", [IN_DIM, NPAIR, 128], f16, isOutput=False)
    w1d = nc.declare_dram_parameter("w1d", [128, NPAIR, 128], f16, isOutput=False)
    b0p = nc.declare_dram_parameter("b0p", [128, NPAIR], f32, isOutput=False)
    b1p = nc.declare_dram_parameter("b1p", [128, NPAIR], f32, isOutput=False)
    out = nc.declare_dram_parameter("out", [NPAIR, 128, B], f16, isOutput=True)

    with ExitStack() as ctx:
        tc = ctx.enter_context(tile.TileContext(nc))
        singles = ctx.enter_context(tc.tile_pool(name="singles", bufs=1))
        hpool = ctx.enter_context(tc.tile_pool(name="hpool", bufs=4))
        opool = ctx.enter_context(tc.tile_pool(name="opool", bufs=5))
        pspool = ctx.enter_context(tc.tile_pool(name="psum", bufs=2, space="PSUM"))

        # Startup-critical loads first: xt + w0 chunk 0 gate mm0(0), b0
        # gates ACT(0), w1 chunk 0 gates mm1(0) two slots later.
        xsb = singles.tile([IN_DIM, B], f16)
        nc.gpsimd.dma_start(out=xsb, in_=xt[:])
        w0sb = singles.tile([IN_DIM, NPAIR, 128], f16)
        w1sb = singles.tile([128, NPAIR, 128], f16)
        # Small head chunks so pair 0 can start ASAP, bulk behind.
        edges = [0, 8, 40, 72, 104, NPAIR]
        b0sb = singles.tile([128, NPAIR], f32)
        b1sb = singles.tile([128, NPAIR], f32)
        nc.gpsimd.dma_start(
            out=w0sb[:, edges[0] : edges[1], :], in_=w0c[:, edges[0] : edges[1], :]
        )
        nc.gpsimd.dma_start(out=b0sb, in_=b0p[:])
        nc.gpsimd.dma_start(
            out=w1sb[:, edges[0] : edges[1], :], in_=w1d[:, edges[0] : edges[1], :]
        )
        nc.gpsimd.dma_start(out=b1sb, in_=b1p[:])
        for c in range(1, len(edges) - 1):
            sl = slice(edges[c], edges[c + 1])
            nc.gpsimd.dma_start(out=w0sb[:, sl, :], in_=w0c[:, sl, :])
            nc.gpsimd.dma_start(out=w1sb[:, sl, :], in_=w1d[:, sl, :])

        hs_live = {}
        for t in range(NPAIR + LAG):
            if t < NPAIR:
                hp = pspool.tile([128, B], f32, tag="hps", name=f"hp{t}")
                nc.tensor.matmul(
                    hp[:, 0:512], w0sb[:, t, :], xsb[:, 0:512],
                    start=True, stop=True,
                )
                mm = nc.tensor.matmul(
                    hp[:, 512:1024], w0sb[:, t, :], xsb[:, 512:1024],
                    start=True, stop=True,
                )
                mm.ins.ldweights = False  # stationary unchanged from half 0
                hs = hpool.tile([128, B], f16, tag="h", name=f"hs{t}")
                nc.scalar.activation(
                    out=hs,
                    in_=hp,
                    func=mybir.ActivationFunctionType.Prelu,
                    bias=b0sb[:, t : t + 1],
                    scale=1.0,
                    alpha=NEG_SLOPE,
                )
                hs_live[t] = hs
            tp = t - LAG
            if tp >= 0:
                hs = hs_live.pop(tp)
                op = pspool.tile([128, B], f32, tag="ops", name=f"op{tp}")
                nc.tensor.matmul(
                    op[:, 0:512], w1sb[:, tp, :], hs[:, 0:512],
                    start=True, stop=True,
                )
                mm = nc.tensor.matmul(
                    op[:, 512:1024], w1sb[:, tp, :], hs[:, 512:1024],
                    start=True, stop=True,
                )
                mm.ins.ldweights = False  # stationary unchanged from half 0
                osb = opool.tile([128, B], f16, tag="o", name=f"os{tp}")
                if tp % ACT_EVAC_MOD == ACT_EVAC_MOD - 1:
                    # DVE is the steady-state critical engine; shift a
                    # slice of the bias-evacuation load onto ACT.
                    nc.scalar.activation(
                        out=osb,
                        in_=op,
                        func=mybir.ActivationFunctionType.Identity,
                        bias=b1sb[:, tp : tp + 1],
                        scale=1.0,
                    )
                else:
                    nc.vector.tensor_scalar_add(
                        osb[:, 0:512], op[:, 0:512], b1sb[:, tp : tp + 1]
                    )
                    nc.vector.tensor_scalar_add(
                        osb[:, 512:1024], op[:, 512:1024], b1sb[:, tp : tp + 1]
                    )
                nc.sync.dma_start(out=out[tp], in_=osb)

    _dedupe_ldweights(nc)
    _split_multi_waits(nc)
    _NC_CACHE = nc
    return nc


def _dedupe_ldweights(nc):
    """The two half-batch matmuls of each (pair, layer) share one
    stationary operand, but tile emits an InstLdweights per matmul.  The
    PE array keeps its weights across matmuls, so a LDWEIGHTS whose AP is
    identical to the previous one on the PE queue is redundant — drop it
    (migrating its semaphore info onto the next PE instruction, where the
    waits still guard the same matmul and the updates fire no earlier
    than before)."""
    import json

    def ldw_key(inst):
        try:
            return mybir.instruction_to_pretty_json_string(inst)
        except Exception:
            return None

    def strip_name(js):
        d = json.loads(js)
        d.pop("name", None)
        d.pop("debug", None)
        d.pop("sync_info", None)
        return json.dumps(d, sort_keys=True)

    n = 0
    for f in nc.m.functions:
        for bb in f.blocks:
            prev_key = None
            out_insts = []
            pending_sync = None
            for inst in bb.instructions:
                eng = getattr(inst, "engine", None)
                if isinstance(inst, mybir.InstLdweights):
                    key = ldw_key(inst)
                    key = strip_name(key) if key else None
                    if key is not None and key == prev_key:
                        si = inst.sync_info
                        if si is not None and (si.on_wait or si.on_update):
                            pending_sync = si
                        n += 1
                        continue  # drop duplicate
                    prev_key = key
                elif eng == mybir.EngineType.PE and pending_sync is not None:
                    si = inst.sync_info
                    waits = list(pending_sync.on_wait or [])
                    upds = list(pending_sync.on_update or [])
                    if si is not None:
                        waits += list(si.on_wait or [])
                        upds += list(si.on_update or [])
                    inst.sync_info = mybir.SyncInfo(on_wait=waits, on_update=upds)
                    pending_sync = None
                out_insts.append(inst)
            assert pending_sync is None, "dangling sync from dropped ldweights"
            bb.instructions = out_insts
    return n


def _hoist_ldweights(nc):
    """Swap each LDWEIGHTS with the PE matmul directly before it, so the
    weight load issues while that matmul still streams (drain overlap)
    instead of serializing after it.  Safe only if the PE double-buffers
    weights (matmul keeps the set it was issued with); correctness is
    checked by the test harness."""
    for f in nc.m.functions:
        for bb in f.blocks:
            insts = bb.instructions
            pe_idx = [
                k
                for k, inst in enumerate(insts)
                if getattr(inst, "engine", None) == mybir.EngineType.PE
            ]
            changed = False
            for pos, k in enumerate(pe_idx):
                inst = insts[k]
                if not isinstance(inst, mybir.InstLdweights):
                    continue
                if pos == 0:
                    continue
                kprev = pe_idx[pos - 1]
                if not isinstance(insts[kprev], mybir.InstMatmult):
                    continue
                insts[k], insts[kprev] = insts[kprev], insts[k]
                changed = True
            if changed:
                bb.instructions = insts
    return nc


def _split_multi_waits(nc):
    """Walrus in this toolchain allows at most ONE semaphore wait per
    instruction.  Hoist all but the last wait of any multi-wait
    instruction onto same-engine NoOp carriers inserted directly before
    it — semantically identical (engine queues are in-order) and each
    carrier holds a single wait."""
    import bass_rust

    n = 0
    for f in nc.m.functions:
        for bb in f.blocks:
            out_insts = []
            changed = False
            for inst in bb.instructions:
                si = inst.sync_info
                waits = list(si.on_wait) if si is not None and si.on_wait else []
                if len(waits) > 1:
                    changed = True
                    for w in waits[:-1]:
                        nop = bass_rust.InstNoOp(
                            name=f"{inst.name}-sw{n}", engine=inst.engine
                        )
                        n += 1
                        nop.sync_info = mybir.SyncInfo(on_wait=[w], on_update=[])
                        out_insts.append(nop)
                    inst.sync_info = mybir.SyncInfo(
                        on_wait=[waits[-1]],
                        on_update=list(si.on_update) if si.on_update else [],
                    )
                out_insts.append(inst)
            if changed:
                bb.instructions = out_insts
    return nc


def _prepare_in_maps(x, W0, b0, W1, b1):
    x = np.asarray(x, dtype=np.float32)
    xt = np.ascontiguousarray(x.T).astype(np.float16)  # (64, 1024)
    in_maps = []
    for c in range(N_CORES):
        sl = slice(c * GPC, (c + 1) * GPC)
        W0c = np.asarray(W0[sl], dtype=np.float32)  # (256, 64, 64) [g, j, k]
        W1c = np.asarray(W1[sl], dtype=np.float32)
        # Layer 0 compact: w0c[k, t, q*64+j] = W0[2t+q, j, k]
        w0ck = np.ascontiguousarray(
            W0c.reshape(NPAIR, 2, D1, IN_DIM)
            .transpose(3, 0, 1, 2)
            .reshape(IN_DIM, NPAIR, 128)
        ).astype(np.float16)
        # Layer 1 block-diagonal: w1d[(qr,k), t, (qc,j)] = [qr==qc]*W1[2t+qc, j, k]
        base = W1c.reshape(NPAIR, 2, D2, D1)  # [t, q, j, k]
        w1dk = np.zeros((2, D1, NPAIR, 2, D2), dtype=np.float32)
        for q in range(2):
            w1dk[q, :, :, q, :] = base[:, q, :, :].transpose(2, 0, 1)  # [k, t, j]
        w1dk = np.ascontiguousarray(w1dk.reshape(128, NPAIR, 128)).astype(np.float16)
        b0pc = np.ascontiguousarray(
            np.asarray(b0[sl], dtype=np.float32).reshape(NPAIR, 128).T
        )  # (128, NPAIR)
        b1pc = np.ascontiguousarray(
            np.asarray(b1[sl], dtype=np.float32).reshape(NPAIR, 128).T
        )
        in_maps.append(
            {"xt": xt, "w0c": w0ck, "w1d": w1dk, "b0p": b0pc, "b1p": b1pc}
        )
    return in_maps


def _postprocess(results):
    outs = []
    for c in range(N_CORES):
        o = results[c]["out"]  # (NPAIR, 128, B) fp16 = [t, q*64+j, b]
        o = (
            o.reshape(NPAIR, 2, 64, B)
            .transpose(3, 0, 1, 2)
            .reshape(B, GPC, D2)
            .astype(np.float32)
        )
        outs.append(o)
    return np.ascontiguousarray(np.concatenate(outs, axis=1))


def _run(inputs, trace=False):
    nc = _build()
    in_maps = _prepare_in_maps(**inputs)
    res = run_bass_kernel_spmd(
        nc, in_maps, core_ids=list(range(N_CORES)), trace=trace
    )
    return _postprocess(res.results), res


def kernel(x, W0, b0, W1, b1):
    out, _ = _run({"x": x, "W0": W0, "b0": b0, "W1": W1, "b1": b1})
    return out


# revision 18
# speedup vs baseline: 1.0832x; 1.0832x over previous
"""Grouped 2-layer MLP (ConvNN) Trainium2 kernel.

Math (per group g of SIZE=2048):
    h[b,g,:]   = LeakyReLU_0.2(W0[g] @ x[b] + b0[g])     (64 -> 64)
    out[b,g,:] = W1[g] @ h[b,g,:] + b1[g]                (64 -> 64)

Strategy:
  - Shard the group axis over 8 cores (256 groups/core = 128 pairs),
    fully independent, no collectives.
  - Groups processed in pairs stacked on the 128 partitions.  Layer 0
    contracts over x (shared by all groups): compact [K=64, M=128]
    stationary, no zeros.  Layer 1 contracts per group: host-built
    128x128 block-diagonal stationary (the zero quadrants cost nothing,
    matmul time is N-driven).
  - Weights DMA straight into their final SBUF layout (no on-chip block
    building).  Small head chunks + critical-first ordering let pair 0
    start ~5us after the SWDGE preamble; bulk chunks stream behind.
  - Per pair: h and out live in [128, 1024] fp32 PSUM tiles (2 banks
    each; 2 bufs per tag fills all 8 banks).  One ACT Prelu (+b0, fp16
    out) evacuates h; one DVE tensor_scalar (+b1, fp16 out) evacuates
    out.  Single 1024-elem instructions amortize the fixed ACT (352cyc)
    and DVE (~210ns) per-instruction overheads; splitting them measured
    slower (more PE-visible sem traffic).
  - Explicit software pipeline (LAG=3): iteration t issues layer-0 of
    pair t and layer-1 of pair t-3, so the PE never waits on the same
    pair's ACT evacuation and streams matmuls back-to-back at the
    426ns/512-row rate this part sustains (PE clock is pinned at
    1.2 GHz here: fp16 and bf16 measured identical, HAM never
    un-throttles).  That makes the kernel PE-streaming-bound:
    512 matmuls x 427ns + 256 weight swaps x ~135ns drain ~= 252us.
  - A post-pass dedupes the per-matmul LDWEIGHTS (the two half-batch
    matmuls of a (pair, layer) share one stationary), halving weight
    loads.  NOTE: the PE weight state is most-recent-LDWEIGHTS-wins --
    reordering a LDWEIGHTS above a consuming matmul breaks numerics
    (verified), and each LDWEIGHTS serializes on the prior matmul's
    drain, which is the remaining ~230ns/slot of overhead.
  - Output is written fp16 as [pair, 128, B] (256KB contiguous DMA per
    pair on the HWDGE/sync ring, kept free of input traffic) and
    un-transposed/cast to fp32 on the host.  fp16 output halves the
    dominant HBM write (64->32MB/core); rel err ~5e-4 vs the 2e-2 gate.

History: baseline 500us -> 277us (fp16 out + big-tile evac + pipeline)
-> 269us (DMA head chunks, LDWEIGHTS dedupe).  Floor analysis: PE
streaming 218us + ldw drains 34us + ramp ~12us + epilogue ~11us.
"""

from contextlib import ExitStack

import numpy as np

import concourse.bass as bass
import concourse.mybir as mybir
import concourse.tile as tile
from concourse.bass_utils import run_bass_kernel_spmd

B = 1024
IN_DIM = 64
SIZE = 2048
D1 = 64
D2 = 64
NEG_SLOPE = 0.2
N_CORES = 8
GPC = SIZE // N_CORES  # 256 groups per core
NPAIR = GPC // 2  # 128 group-pairs per core
WCH = 4  # weight DMA chunks per layer
LAG = 3  # software-pipeline distance between layer 0 and layer 1

_NC_CACHE = None


def _build():
    global _NC_CACHE
    if _NC_CACHE is not None:
        return _NC_CACHE

    f32 = mybir.dt.float32
    f16 = mybir.dt.float16

    nc = bass.Bass()
    xt = nc.declare_dram_parameter("xt", [IN_DIM, B], f16, isOutput=False)
    w0c = nc.declare_dram_parameter("w0c", [IN_DIM, NPAIR, 128], f16, isOutput=False)
    w1d = nc.declare_dram_parameter("w1d", [128, NPAIR, 128], f16, isOutput=False)
    b0p = nc.declare_dram_parameter("b0p", [128, NPAIR], f32, isOutput=False)
    b1p = nc.declare_dram_parameter("b1p", [128, NPAIR], f32, isOutput=False)
    out = nc.declare_dram_parameter("out", [NPAIR, 128, B], f16, isOutput=True)

    with ExitStack() as ctx:
        tc = ctx.enter_context(tile.TileContext(nc))
        singles = ctx.enter_context(tc.tile_pool(name="singles", bufs=1))
        hpool = ctx.enter_context(tc.tile_pool(name="hpool", bufs=4))
        opool = ctx.enter_context(tc.tile_pool(name="opool", bufs=5))
        pspool = ctx.enter_context(tc.tile_pool(name="psum", bufs=2, space="PSUM"))

        # Startup-critical loads first: xt + w0 chunk 0 gate mm0(0), b0
        # gates ACT(0), w1 chunk 0 gates mm1(0) two slots later.
        xsb = singles.tile([IN_DIM, B], f16)
        nc.gpsimd.dma_start(out=xsb, in_=xt[:])
        w0sb = singles.tile([IN_DIM, NPAIR, 128], f16)
        w1sb = singles.tile([128, NPAIR, 128], f16)
        # Small head chunks so pair 0 can start ASAP, bulk behind.
        edges = [0, 8, 40, 72, 104, NPAIR]
        b0sb = singles.tile([128, NPAIR], f32)
        b1sb = singles.tile([128, NPAIR], f32)
        nc.gpsimd.dma_start(
            out=w0sb[:, edges[0] : edges[1], :], in_=w0c[:, edges[0] : edges[1], :]
        )
        nc.gpsimd.dma_start(out=b0sb, in_=b0p[:])
        nc.gpsimd.dma_start(
            out=w1sb[:, edges[0] : edges[1], :], in_=w1d[:, edges[0] : edges[1], :]
        )
        nc.gpsimd.dma_start(out=b1sb, in_=b1p[:])
        for c in range(1, len(edges) - 1):
            sl = slice(edges[c], edges[c + 1])
            nc.gpsimd.dma_start(out=w0sb[:, sl, :], in_=w0c[:, sl, :])
            nc.gpsimd.dma_start(out=w1sb[:, sl, :], in_=w1d[:, sl, :])

        hs_live = {}
        for t in range(NPAIR + LAG):
            if t < NPAIR:
                hp = pspool.tile([128, B], f32, tag="hps", name=f"hp{t}")
                nc.tensor.matmul(
                    hp[:, 0:512], w0sb[:, t, :], xsb[:, 0:512],
                    start=True, stop=True,
                )
                mm = nc.tensor.matmul(
                    hp[:, 512:1024], w0sb[:, t, :], xsb[:, 512:1024],
                    start=True, stop=True,
                )
                mm.ins.ldweights = False  # stationary unchanged from half 0
                hs = hpool.tile([128, B], f16, tag="h", name=f"hs{t}")
                nc.scalar.activation(
                    out=hs,
                    in_=hp,
                    func=mybir.ActivationFunctionType.Prelu,
                    bias=b0sb[:, t : t + 1],
                    scale=1.0,
                    alpha=NEG_SLOPE,
                )
                hs_live[t] = hs
            tp = t - LAG
            if tp >= 0:
                hs = hs_live.pop(tp)
                op = pspool.tile([128, B], f32, tag="ops", name=f"op{tp}")
                nc.tensor.matmul(
                    op[:, 0:512], w1sb[:, tp, :], hs[:, 0:512],
                    start=True, stop=True,
                )
                mm = nc.tensor.matmul(
                    op[:, 512:1024], w1sb[:, tp, :], hs[:, 512:1024],
                    start=True, stop=True,
                )
                mm.ins.ldweights = False  # stationary unchanged from half 0
                osb = opool.tile([128, B], f16, tag="o", name=f"os{tp}")
                nc.vector.tensor_scalar_add(osb, op, b1sb[:, tp : tp + 1])
                nc.sync.dma_start(out=out[tp], in_=osb)

    _dedupe_ldweights(nc)
    _split_multi_waits(nc)
    _NC_CACHE = nc
    return nc


def _dedupe_ldweights(nc):
    """The two half-batch matmuls of each (pair, layer) share one
    stationary operand, but tile emits an InstLdweights per matmul.  The
    PE array keeps its weights across matmuls, so a LDWEIGHTS whose AP is
    identical to the previous one on the PE queue is redundant — drop it
    (migrating its semaphore info onto the next PE instruction, where the
    waits still guard the same matmul and the updates fire no earlier
    than before)."""
    import json

    def ldw_key(inst):
        try:
            return mybir.instruction_to_pretty_json_string(inst)
        except Exception:
            return None

    def strip_name(js):
        d = json.loads(js)
        d.pop("name", None)
        d.pop("debug", None)
        d.pop("sync_info", None)
        return json.dumps(d, sort_keys=True)

    n = 0
    for f in nc.m.functions:
        for bb in f.blocks:
            prev_key = None
            out_insts = []
            pending_sync = None
            for inst in bb.instructions:
                eng = getattr(inst, "engine", None)
                if isinstance(inst, mybir.InstLdweights):
                    key = ldw_key(inst)
                    key = strip_name(key) if key else None
                    if key is not None and key == prev_key:
                        si = inst.sync_info
                        if si is not None and (si.on_wait or si.on_update):
                            pending_sync = si
                        n += 1
                        continue  # drop duplicate
                    prev_key = key
                elif eng == mybir.EngineType.PE and pending_sync is not None:
                    si = inst.sync_info
                    waits = list(pending_sync.on_wait or [])
                    upds = list(pending_sync.on_update or [])
                    if si is not None:
                        waits += list(si.on_wait or [])
                        upds += list(si.on_update or [])
                    inst.sync_info = mybir.SyncInfo(on_wait=waits, on_update=upds)
                    pending_sync = None
                out_insts.append(inst)
            assert pending_sync is None, "dangling sync from dropped ldweights"
            bb.instructions = out_insts
    return n


def _split_multi_waits(nc):
    """Walrus in this toolchain allows at most ONE semaphore wait per
    instruction.  Hoist all but the last wait of any multi-wait
    instruction onto same-engine NoOp carriers inserted directly before
    it — semantically identical (engine queues are in-order) and each
    carrier holds a single wait."""
    import bass_rust

    n = 0
    for f in nc.m.functions:
        for bb in f.blocks:
            out_insts = []
            changed = False
            for inst in bb.instructions:
                si = inst.sync_info
                waits = list(si.on_wait) if si is not None and si.on_wait else []
                if len(waits) > 1:
                    changed = True
                    for w in waits[:-1]:
                        nop = bass_rust.InstNoOp(
                            name=f"{inst.name}-sw{n}", engine=inst.engine
                        )
                        n += 1
                        nop.sync_info = mybir.SyncInfo(on_wait=[w], on_update=[])
                        out_insts.append(nop)
                    inst.sync_info = mybir.SyncInfo(
                        on_wait=[waits[-1]],
                        on_update=list(si.on_update) if si.on_update else [],
                    )
                out_insts.append(inst)
            if changed:
                bb.instructions = out_insts
    return nc


def _prepare_in_maps(x, W0, b0, W1, b1):
    x = np.asarray(x, dtype=np.float32)
    xt = np.ascontiguousarray(x.T).astype(np.float16)  # (64, 1024)
    in_maps = []
    for c in range(N_CORES):
        sl = slice(c * GPC, (c + 1) * GPC)
        W0c = np.asarray(W0[sl], dtype=np.float32)  # (256, 64, 64) [g, j, k]
        W1c = np.asarray(W1[sl], dtype=np.float32)
        # Layer 0 compact: w0c[k, t, q*64+j] = W0[2t+q, j, k]
        w0ck = np.ascontiguousarray(
            W0c.reshape(NPAIR, 2, D1, IN_DIM)
            .transpose(3, 0, 1, 2)
            .reshape(IN_DIM, NPAIR, 128)
        ).astype(np.float16)
        # Layer 1 block-diagonal: w1d[(qr,k), t, (qc,j)] = [qr==qc]*W1[2t+qc, j, k]
        base = W1c.reshape(NPAIR, 2, D2, D1)  # [t, q, j, k]
        w1dk = np.zeros((2, D1, NPAIR, 2, D2), dtype=np.float32)
        for q in range(2):
            w1dk[q, :, :, q, :] = base[:, q, :, :].transpose(2, 0, 1)  # [k, t, j]
        w1dk = np.ascontiguousarray(w1dk.reshape(128, NPAIR, 128)).astype(np.float16)
        b0pc = np.ascontiguousarray(
            np.asarray(b0[sl], dtype=np.float32).reshape(NPAIR, 128).T
        )  # (128, NPAIR)
        b1pc = np.ascontiguousarray(
            np.asarray(b1[sl], dtype=np.float32).reshape(NPAIR, 128).T
        )
        in_maps.append(
            {"xt": xt, "w0c": w0ck, "w1d": w1dk, "b0p": b0pc, "b1p": b1pc}
        )
    return in_maps


def _postprocess(results):
    outs = []
    for c in range(N_CORES):
        o = results[c]["out"]  # (NPAIR, 128, B) fp16 = [t, q*64+j, b]
        o = (
            o.reshape(NPAIR, 2, 64, B)
            .transpose(3, 0, 1, 2)
            .reshape(B, GPC, D2)
            .astype(np.float32)
        )
        outs.append(o)
    return np.ascontiguousarray(np.concatenate(outs, axis=1))


def _run(inputs, trace=False):
    nc = _build()
    in_maps = _prepare_in_maps(**inputs)
    res = run_bass_kernel_spmd(
        nc, in_maps, core_ids=list(range(N_CORES)), trace=trace
    )
    return _postprocess(res.results), res


def kernel(x, W0, b0, W1, b1):
    out, _ = _run({"x": x, "W0": W0, "b0": b0, "W1": W1, "b1": b1})
    return out
